# revision 26
# baseline (speedup 1.0000x reference)
"""MultiHeadLatentAttention (MLA) Trainium2 kernel — 8-core SPMD, tensor-parallel over heads.

v3 strategy (per core c, owning heads {2c, 2c+1} and tokens [512c, 512c+512)):
  - Q path fused on host: Wq_h = wq_up_h @ diag(q_norm_w) @ wq_down; alpha_t needs
    ||x @ wq_down.T||^2 over all 1536 ranks: each core computes a 192-rank shard of
    the sum of squares (partition-sum via ones-matmul so the Pool engine stays free
    for collectives) and a tiny [1, 2048] AllReduce per batch completes it.
  - KV path token-sharded: each core computes kv_c, beta, and rotated k_rope for its
    own 512 tokens only (pass 1), pre-scales kv_c by beta, and one [576, 512] bf16
    AllGather (hidden under the Q front) distributes them; pass 3 then runs the
    per-head kv up-projection (head-sharded weights) over all tokens.
  - All tensor-engine matmuls in bf16 (f32 PSUM accumulation); norm scales and the
    softmax denominator stay f32.
  - Attention in transposed layout S^T[k, q]; diagonal 128-blocks are clipped to the
    causal column range (matmul/exp/mask/accumulate only on valid columns); softmax
    denominator via DVE accumulation + gpsimd partition_all_reduce; no max
    subtraction (scores are O(5), exp is safe in fp32).
  - wo applied per core to its 2 heads; host sums the 8 partial [2048, 4096] outputs.
"""

import math
import numpy as np

import concourse.bacc as bacc
import concourse.mybir as mybir
import concourse.tile as tile
from concourse.bass_utils import run_bass_kernel_spmd

F32 = mybir.dt.float32
BF16 = mybir.dt.bfloat16
NPBF16 = mybir.dt.np(mybir.dt.bfloat16)

N_CORES = 8
HPC = 2               # heads per core
DIM = 2048
NH = 16
QR = 1536
KVR = 512
DN = 128
DR = 64
DV = 128
B = 2
S = 2048
T = B * S
EPS = 1e-6
SCALE = 1.0 / math.sqrt(DN + DR)
ROPE_THETA = 10000.0

TB = 512              # front token block (main loop)
NTB = S // TB         # 4 blocks per batch
TB3 = 256             # pass-3 token block

DCH = DIM // 128      # 16 contraction chunks
RSH = QR // N_CORES   # 192-rank ss shard per core
OWN = 512             # own tokens per core (token-sharded kv path)
AGR = KVR + DR        # rows per core in the kv/kr allgather block (576)

_BUILD_CACHE = {}

import os
V4_LBF16 = os.environ.get("V4_LBF16", "1") == "1"
V4_DVESQ = os.environ.get("V4_DVESQ", "0") == "1"
V4_YBF16 = os.environ.get("V4_YBF16", "1") == "1"
LDT = BF16 if V4_LBF16 else F32
YDT = None


def _build_program(reps=1, phase=None):
    phase = phase or os.environ.get("PROF_PHASE", "full")
    if ("nc", reps, phase) in _BUILD_CACHE:
        return _BUILD_CACHE[("nc", reps, phase)]

    nc = bacc.Bacc(num_devices=N_CORES)

    # ---------------- DRAM I/O ----------------
    xT_d = nc.dram_tensor("xT", [B, DIM, S], BF16, kind="ExternalInput")
    xo_d = nc.dram_tensor("xo", [DIM, OWN], BF16, kind="ExternalInput")
    wqss_d = nc.dram_tensor("wqss", [DIM, RSH], BF16, kind="ExternalInput")
    wq_d = nc.dram_tensor("wq", [DIM, HPC * DN], BF16, kind="ExternalInput")
    wqp_d = nc.dram_tensor("wqp", [DIM, HPC * DR], BF16, kind="ExternalInput")
    wkvd_d = nc.dram_tensor("wkvd", [DIM, KVR], BF16, kind="ExternalInput")
    wkvu_d = nc.dram_tensor("wkvu", [KVR, HPC * (DN + DV)], BF16, kind="ExternalInput")
    wkr_d = nc.dram_tensor("wkr", [DIM, DR], BF16, kind="ExternalInput")
    wo0_d = nc.dram_tensor("wo0", [DV, DIM], BF16, kind="ExternalInput")
    wo1_d = nc.dram_tensor("wo1", [DV, DIM], BF16, kind="ExternalInput")
    ctab_d = nc.dram_tensor("ctab", [128, S], F32, kind="ExternalInput")
    stab_d = nc.dram_tensor("stab", [128, S], F32, kind="ExternalInput")
    ctabo_d = nc.dram_tensor("ctabo", [64, OWN], F32, kind="ExternalInput")
    stabo_d = nc.dram_tensor("stabo", [64, OWN], F32, kind="ExternalInput")
    masks_d = nc.dram_tensor("masks", [128, 128], BF16, kind="ExternalInput")
    ident_d = nc.dram_tensor("ident", [128, 128], BF16, kind="ExternalInput")
    ones_d = nc.dram_tensor("ones", [128, 1], BF16, kind="ExternalInput")

    yT_d = nc.dram_tensor("yT", [DIM, T], BF16 if V4_YBF16 else F32, kind="ExternalOutput")

    # ---------------- internal DRAM scratch ----------------
    qn_s = [[nc.dram_tensor(f"qn_{b}_{h}", [DN, S], BF16) for h in range(HPC)] for b in range(B)]
    qp_s = [nc.dram_tensor(f"qp_{b}", [HPC * DR, S], BF16) for b in range(B)]
    kn_s = [[nc.dram_tensor(f"kn_{b}_{h}", [DN, S], BF16) for h in range(HPC)] for b in range(B)]
    v_s = [[nc.dram_tensor(f"v_{b}_{h}", [S, DV], BF16) for h in range(HPC)] for b in range(B)]
    ssin = [nc.dram_tensor(f"ssin_{b}", [1, S], F32) for b in range(B)]
    ssout = [nc.dram_tensor(f"ssout_{b}", [1, S], F32, addr_space="Shared") for b in range(B)]
    ag_in = nc.dram_tensor("ag_in", [AGR, OWN], BF16)
    ag_out = nc.dram_tensor("ag_out", [N_CORES * AGR, OWN], BF16, addr_space="Shared")

    import concourse.bass_isa as bass_isa
    RADD = bass_isa.ReduceOp.add
    ACTF = mybir.ActivationFunctionType

    with tile.TileContext(nc) as tc:
        with tc.tile_pool(name="wpool", bufs=1) as wp:
            # resident weights / constants
            wq_t = wp.tile([128, DCH, HPC * DN], BF16, tag="wq")
            wqp_t = wp.tile([128, DCH, HPC * DR], BF16, tag="wqp")
            wkvu_t = wp.tile([128, KVR // 128, HPC * (DN + DV)], BF16, tag="wkvu")
            wo_t = [wp.tile([DV, DIM], BF16, tag=f"wo{h}", name=f"wo_t{h}") for h in range(HPC)]
            masks_t = wp.tile([128, 128], BF16, tag="masks")
            nc.sync.dma_start(masks_t[:], masks_d[:])
            ident_t = wp.tile([128, 128], BF16, tag="ident")
            nc.sync.dma_start(ident_t[:], ident_d[:])
            ones_t = wp.tile([128, 1], BF16, tag="ones")
            nc.sync.dma_start(ones_t[:], ones_d[:])
            eps_t = wp.tile([128, 1], F32, tag="eps")
            nc.gpsimd.memset(eps_t[:], EPS)
            ctab_t = wp.tile([128, S], F32, tag="ctab")
            nc.sync.dma_start(ctab_t[:], ctab_d[:])
            stab_t = wp.tile([128, S], F32, tag="stab")
            nc.sync.dma_start(stab_t[:], stab_d[:])
            nc.sync.dma_start(wq_t[:], wq_d.ap().rearrange("(c p) m -> p c m", p=128))
            nc.sync.dma_start(wqp_t[:], wqp_d.ap().rearrange("(c p) m -> p c m", p=128))
            nc.sync.dma_start(wkvu_t[:], wkvu_d.ap().rearrange("(c p) m -> p c m", p=128))
            nc.sync.dma_start(wo_t[0][:], wo0_d[:])
            nc.sync.dma_start(wo_t[1][:], wo1_d[:])

            def sec_pass1():
                # =============== PASS 1: own-token kv_c, beta, k_rope ===============
                with tc.tile_pool(name="p1", bufs=1) as p1, \
                     tc.tile_pool(name="p1ps", bufs=1, space="PSUM") as pp1:
                    wkvd_t = p1.tile([128, DCH, KVR], BF16, tag="wkvd")
                    nc.sync.dma_start(wkvd_t[:], wkvd_d.ap().rearrange("(c p) m -> p c m", p=128))
                    wkr_t = p1.tile([128, DCH, DR], BF16, tag="wkr")
                    nc.sync.dma_start(wkr_t[:], wkr_d.ap().rearrange("(c p) m -> p c m", p=128))
                    ctabo_t = p1.tile([64, OWN], F32, tag="ctabo")
                    nc.sync.dma_start(ctabo_t[:], ctabo_d[:])
                    stabo_t = p1.tile([64, OWN], F32, tag="stabo")
                    nc.sync.dma_start(stabo_t[:], stabo_d[:])
                    xo = p1.tile([128, DCH, OWN], BF16, tag="xo")
                    nc.sync.dma_start(xo[:], xo_d.ap().rearrange("(c p) t -> p c t", p=128))

                    kvc = p1.tile([128, KVR // 128, OWN], BF16, tag="kvc")
                    pss = pp1.tile([1, OWN], F32, tag="pss")
                    for rc in range(KVR // 128):
                        ps_kv = pp1.tile([128, OWN], F32, tag="pkv", bufs=2)
                        for d in range(DCH):
                            nc.tensor.matmul(ps_kv[:], wkvd_t[:, d, rc * 128:(rc + 1) * 128],
                                             xo[:, d, :], start=(d == 0), stop=(d == DCH - 1))
                        nc.vector.tensor_copy(kvc[:, rc, :], ps_kv[:])
                        sq = p1.tile([128, OWN], BF16, tag="sq", bufs=2)
                        if V4_DVESQ:
                            nc.vector.tensor_mul(sq[:], ps_kv[:], ps_kv[:])
                        else:
                            nc.scalar.activation(sq[:], ps_kv[:], ACTF.Square)
                        nc.tensor.matmul(pss[:], ones_t[:], sq[:],
                                         start=(rc == 0), stop=(rc == KVR // 128 - 1))
                    # beta = 1/sqrt(mean+eps) = exp(-0.5*ln(mean+eps)); Ln+Exp share
                    # one ACT table set so the engine never reloads tables.
                    blog = p1.tile([1, OWN], F32, tag="blog")
                    nc.scalar.activation(blog[:], pss[:], ACTF.Ln,
                                         scale=1.0 / KVR, bias=eps_t[0:1, :])
                    brow = p1.tile([1, OWN], F32, tag="brow")
                    nc.scalar.activation(brow[:], blog[:], ACTF.Exp, scale=-0.5)
                    bbc = p1.tile([128, OWN], F32, tag="bbc")
                    nc.gpsimd.partition_broadcast(bbc[:], brow[:])
                    kvcs = p1.tile([128, KVR // 128, OWN], BF16, tag="kvcs")
                    for rc in range(KVR // 128):
                        nc.vector.tensor_mul(kvcs[:, rc, :], kvc[:, rc, :], bbc[:])
                    nc.sync.dma_start(
                        ag_in.ap()[0:KVR, :].rearrange("(c p) t -> p c t", p=128), kvcs[:])

                    # k_rope for own tokens + rotation
                    ps_kr = pp1.tile([64, OWN], F32, tag="pkr")
                    for d in range(DCH):
                        nc.tensor.matmul(ps_kr[:], wkr_t[:, d, :], xo[:, d, :],
                                         start=(d == 0), stop=(d == DCH - 1))
                    tmp = p1.tile([64, OWN], F32, tag="krtmp")
                    nc.vector.tensor_copy(tmp[0:32, :], ps_kr[32:64, :])
                    nc.vector.tensor_copy(tmp[32:64, :], ps_kr[0:32, :])
                    m1 = p1.tile([64, OWN], F32, tag="krm1")
                    nc.vector.tensor_mul(m1[:], ps_kr[:], ctabo_t[:])
                    nc.vector.tensor_mul(tmp[:], tmp[:], stabo_t[:])
                    krr = p1.tile([64, OWN], BF16, tag="krr")
                    nc.vector.tensor_add(krr[:], m1[:], tmp[:])
                    nc.sync.dma_start(ag_in.ap()[KVR:AGR, :], krr[:])

                nc.gpsimd.collective_compute(
                    "AllGather", mybir.AluOpType.bypass,
                    replica_groups=[list(range(N_CORES))],
                    ins=[ag_in.ap()], outs=[ag_out.ap()],
                )

            def sec_main():
                # =============== MAIN LOOP: fused Q front + ss shard ===============
                with tc.tile_pool(name="fpool", bufs=1) as fp, \
                     tc.tile_pool(name="fps", bufs=1, space="PSUM") as fpp:
                    wqss_t = fp.tile([128, DCH, RSH], BF16, tag="wqss")
                    nc.sync.dma_start(wqss_t[:], wqss_d.ap().rearrange("(c p) m -> p c m", p=128))
                    for b in range(B):
                        ss_row = fp.tile([1, S], F32, tag="ss_row")
                        for j in range(NTB):
                            t0 = j * TB
                            xt = fp.tile([128, DCH, TB], BF16, tag="xt", bufs=3)
                            nc.sync.dma_start(
                                xt[:], xT_d.ap()[b, :, t0:t0 + TB].rearrange("(c p) t -> p c t", p=128))

                            # ---- ss shard (raw q_c norm partial) ----
                            ps_a = fpp.tile([128, TB], F32, tag="p128", bufs=3)
                            for d in range(DCH):
                                nc.tensor.matmul(ps_a[:], wqss_t[:, d, 0:128], xt[:, d, :],
                                                 start=(d == 0), stop=(d == DCH - 1))
                            ps_b = fpp.tile([64, TB], F32, tag="p64", bufs=2)
                            for d in range(DCH):
                                nc.tensor.matmul(ps_b[:], wqss_t[:, d, 128:192], xt[:, d, :],
                                                 start=(d == 0), stop=(d == DCH - 1))
                            sq_a = fp.tile([128, TB], BF16, tag="sq_a", bufs=2)
                            sq_b = fp.tile([64, TB], BF16, tag="sq_b", bufs=2)
                            if V4_DVESQ:
                                nc.vector.tensor_mul(sq_a[:], ps_a[:], ps_a[:])
                                nc.vector.tensor_mul(sq_b[:], ps_b[:], ps_b[:])
                            else:
                                nc.scalar.activation(sq_a[:], ps_a[:], ACTF.Square)
                                nc.scalar.activation(sq_b[:], ps_b[:], ACTF.Square)
                            ss_ps = fpp.tile([1, TB], F32, tag="ssp", bufs=2)
                            nc.tensor.matmul(ss_ps[:], ones_t[:], sq_a[:], start=True, stop=False)
                            nc.tensor.matmul(ss_ps[:], ones_t[0:64, :], sq_b[:], start=False, stop=True)
                            nc.vector.tensor_copy(ss_row[0:1, t0:t0 + TB], ss_ps[:])

                            # ---- Qn raw (2 heads) ----
                            for h in range(HPC):
                                ps_qn = fpp.tile([128, TB], F32, tag="p128", bufs=3)
                                for d in range(DCH):
                                    nc.tensor.matmul(ps_qn[:], wq_t[:, d, h * DN:(h + 1) * DN],
                                                     xt[:, d, :], start=(d == 0), stop=(d == DCH - 1))
                                qstg = fp.tile([128, TB], BF16, tag="qstg", bufs=2)
                                nc.vector.tensor_copy(qstg[:], ps_qn[:])
                                nc.sync.dma_start(qn_s[b][h].ap()[:, t0:t0 + TB], qstg[:])

                            # ---- Qp raw (2 heads stacked) + rope ----
                            ps_qp = fpp.tile([128, TB], F32, tag="p128", bufs=3)
                            for d in range(DCH):
                                nc.tensor.matmul(ps_qp[:], wqp_t[:, d, :], xt[:, d, :],
                                                 start=(d == 0), stop=(d == DCH - 1))
                            qtmp = fp.tile([128, TB], F32, tag="qptmp", bufs=2)
                            for h in range(HPC):
                                o = h * 64
                                nc.vector.tensor_copy(qtmp[o:o + 32, :], ps_qp[o + 32:o + 64, :])
                                nc.vector.tensor_copy(qtmp[o + 32:o + 64, :], ps_qp[o:o + 32, :])
                            qm1 = fp.tile([128, TB], F32, tag="qpm1", bufs=2)
                            nc.vector.tensor_mul(qm1[:], ps_qp[:], ctab_t[:, t0:t0 + TB])
                            nc.vector.tensor_mul(qtmp[:], qtmp[:], stab_t[:, t0:t0 + TB])
                            qrot = fp.tile([128, TB], BF16, tag="qrot", bufs=2)
                            nc.vector.tensor_add(qrot[:], qm1[:], qtmp[:])
                            nc.sync.dma_start(qp_s[b].ap()[:, t0:t0 + TB], qrot[:])

                        nc.sync.dma_start(ssin[b][:], ss_row[:])
                        nc.gpsimd.collective_compute(
                            "AllReduce", mybir.AluOpType.add,
                            replica_groups=[list(range(N_CORES))],
                            ins=[ssin[b][:]], outs=[ssout[b][:]],
                        )

            def sec_pass3():
                # =============== PASS 3: kv up-projection (all tokens) ===============
                with tc.tile_pool(name="p3", bufs=1) as p3, \
                     tc.tile_pool(name="p3ps", bufs=1, space="PSUM") as pp3:
                    for g2 in range(2 * N_CORES):
                        g, half = g2 // 2, g2 % 2
                        b, t0 = g // 4, (g % 4) * 512 + half * 256
                        kvg = p3.tile([128, KVR // 128, TB3], BF16, tag="kvg", bufs=3)
                        nc.sync.dma_start(
                            kvg[:],
                            ag_out.ap()[AGR * g: AGR * g + KVR, half * TB3:(half + 1) * TB3]
                            .rearrange("(c p) t -> p c t", p=128))
                        for m in range(4):  # 0: K h0, 1: V h0, 2: K h1, 3: V h1
                            h, is_v = m // 2, m % 2
                            ps_up = pp3.tile([128, TB3], F32, tag="pup", bufs=4)
                            for rc in range(KVR // 128):
                                nc.tensor.matmul(ps_up[:], wkvu_t[:, rc, m * 128:(m + 1) * 128],
                                                 kvg[:, rc, :], start=(rc == 0), stop=(rc == 3))
                            stg = p3.tile([128, TB3], BF16, tag="stg_up", bufs=2)
                            nc.vector.tensor_copy(stg[:], ps_up[:])
                            if not is_v:
                                nc.sync.dma_start(kn_s[b][h].ap()[:, t0:t0 + TB3], stg[:])
                            else:
                                for c2 in range(TB3 // 128):
                                    tps = pp3.tile([128, 128], BF16, tag="ptp", bufs=2)
                                    nc.tensor.transpose(tps[:], stg[:, c2 * 128:(c2 + 1) * 128], ident_t[:])
                                    vn = p3.tile([128, 128], BF16, tag="vn", bufs=2)
                                    nc.vector.tensor_copy(vn[:], tps[:])
                                    nc.sync.dma_start(
                                        v_s[b][h].ap()[t0 + c2 * 128: t0 + (c2 + 1) * 128, :], vn[:])

            def sec_attn():
                # ======================= ATTENTION PHASE =======================
                with tc.tile_pool(name="apool", bufs=1) as ap, \
                     tc.tile_pool(name="aps", bufs=1, space="PSUM") as app:
                  for b in range(B):
                      # alpha = 1/sqrt(ss/QR + eps) = exp(-0.5*ln(...)): stays in the
                      # exp table set (no ACT table reload), broadcast to 128 partitions
                      ssr = ap.tile([1, S], F32, tag="ssr")
                      nc.sync.dma_start(ssr[:], ssout[b][:])
                      alog = ap.tile([1, S], F32, tag="alog")
                      nc.scalar.activation(alog[:], ssr[:], ACTF.Ln,
                                           scale=1.0 / QR, bias=eps_t[0:1, :])
                      arow = ap.tile([1, S], F32, tag="arow")
                      nc.scalar.activation(arow[:], alog[:], ACTF.Exp, scale=-0.5)
                      abc = ap.tile([128, S], F32, tag="abc")
                      nc.gpsimd.partition_broadcast(abc[:], arow[:])

                      # k_rope duplicated on both partition halves so the two heads'
                      # K=64 rope-score matmuls can run row-packed (tile_position
                      # (0,0) / (64,0)) concurrently in disjoint array halves.
                      kr_sb = ap.tile([128, S], BF16, tag="kr_sb")
                      for ck in range(4):
                          g = 4 * b + ck
                          nc.sync.dma_start(kr_sb[0:64, ck * 512:(ck + 1) * 512],
                                            ag_out.ap()[AGR * g + KVR: AGR * (g + 1), :])
                          nc.sync.dma_start(kr_sb[64:128, ck * 512:(ck + 1) * 512],
                                            ag_out.ap()[AGR * g + KVR: AGR * (g + 1), :])

                      kn_sb = [ap.tile([128, S], BF16, tag=f"kn_sb{h}", name=f"kn_sb{h}", bufs=2) for h in range(HPC)]
                      v_sb = [ap.tile([128, S // 128, DV], BF16, tag=f"v_sb{h}", name=f"v_sb{h}", bufs=2) for h in range(HPC)]
                      for h in range(HPC):
                          for ck in range(4):
                              nc.sync.dma_start(kn_sb[h][:, ck * 512:(ck + 1) * 512],
                                                kn_s[b][h].ap()[:, ck * 512:(ck + 1) * 512])
                              nc.sync.dma_start(
                                  v_sb[h][:, ck * 4:(ck + 1) * 4, :],
                                  v_s[b][h].ap()[ck * 512:(ck + 1) * 512, :].rearrange("(c p) v -> p c v", p=128))

                      for qt in range(4):
                          q0 = qt * 512
                          nkc = 4 * (qt + 1)
                          # q tiles for both heads up front; qp holds h0|h1 stacked on
                          # the two partition halves (matches kr_sb duplication).
                          qn_sc = []
                          for h in range(HPC):
                              qn_t = ap.tile([128, 512], BF16, tag="qn_t", bufs=3)
                              nc.sync.dma_start(qn_t[:], qn_s[b][h].ap()[:, q0:q0 + 512])
                              qsc = ap.tile([128, 512], BF16, tag="qn_sc", bufs=3)
                              nc.vector.tensor_mul(qsc[:], qn_t[:], abc[:, q0:q0 + 512])
                              qn_sc.append(qsc)
                          qp_t = ap.tile([128, 512], BF16, tag="qp_t", bufs=2)
                          nc.sync.dma_start(qp_t[:], qp_s[b].ap()[:, q0:q0 + 512])
                          qp_sc = ap.tile([128, 512], BF16, tag="qp_sc", bufs=2)
                          nc.vector.tensor_mul(qp_sc[:], qp_t[:], abc[:, q0:q0 + 512])

                          O = [app.tile([128, 512], F32, tag=f"pO{h}", name=f"pO{h}", bufs=1)
                               for h in range(HPC)]
                          l_acc = ap.tile([128, HPC, 512], LDT, tag="l_acc", bufs=2)
                          for kc in range(nkc):
                              k0 = kc * 128
                              mi = kc - 4 * qt      # >= 0 on the diagonal blocks
                              c0 = max(0, mi) * 128  # first causally-valid q column
                              # both heads' scores in one 2-bank PSUM tile (h on dim 1)
                              s_ps = app.tile([128, HPC, 512], F32, tag="ps_s", bufs=2)
                              nc.tensor.matmul(s_ps[:, 0, c0:], kn_sb[0][:, k0:k0 + 128],
                                               qn_sc[0][:, c0:], start=True, stop=False)
                              nc.tensor.matmul(s_ps[:, 1, c0:], kn_sb[1][:, k0:k0 + 128],
                                               qn_sc[1][:, c0:], start=True, stop=False)
                              # packed rope scores: h0 in array rows 0-63, h1 in 64-127
                              nc.tensor.matmul(s_ps[:, 0, c0:], kr_sb[0:64, k0:k0 + 128],
                                               qp_sc[0:64, c0:], start=False, stop=True,
                                               tile_position=(0, 0))
                              nc.tensor.matmul(s_ps[:, 1, c0:], kr_sb[64:128, k0:k0 + 128],
                                               qp_sc[64:128, c0:], start=False, stop=True,
                                               tile_position=(64, 0))
                              P = ap.tile([128, HPC, 512], BF16, tag="P", bufs=4)
                              nc.scalar.activation(P[:, :, c0:], s_ps[:, :, c0:], ACTF.Exp,
                                                   scale=SCALE)
                              if mi >= 0:
                                  nc.vector.tensor_mul(P[:, 0, c0:c0 + 128], P[:, 0, c0:c0 + 128],
                                                       masks_t[:])
                                  nc.vector.tensor_mul(P[:, 1, c0:c0 + 128], P[:, 1, c0:c0 + 128],
                                                       masks_t[:])
                              if kc == 0:
                                  nc.vector.tensor_copy(l_acc[:], P[:])
                              else:
                                  nc.vector.tensor_add(l_acc[:, :, c0:], l_acc[:, :, c0:], P[:, :, c0:])
                              nc.tensor.matmul(O[0][:, c0:], v_sb[0][:, kc, :], P[:, 0, c0:],
                                               start=(kc == 0), stop=(kc == nkc - 1))
                              nc.tensor.matmul(O[1][:, c0:], v_sb[1][:, kc, :], P[:, 1, c0:],
                                               start=(kc == 0), stop=(kc == nkc - 1))
                          l_bc = ap.tile([128, HPC, 512], F32, tag="l_bc", bufs=2)
                          nc.gpsimd.partition_all_reduce(l_bc[:], l_acc[:], channels=128, reduce_op=RADD)
                          l_rec = ap.tile([128, HPC, 512], F32, tag="l_rec", bufs=2)
                          nc.vector.reciprocal_approx_fast(l_rec[:], l_bc[:])
                          out_q = []
                          for h in range(HPC):
                              out_sb = ap.tile([128, 512], BF16, tag="out_sb", bufs=4)
                              nc.vector.tensor_mul(out_sb[:], O[h][:], l_rec[:, h, :])
                              out_q.append(out_sb)

                          # wo for this (b, qt): overlaps the next qt's exp/DVE work;
                          # evictions split between ACT and DVE to balance engines
                          for dm in range(DCH):
                              y_ps = app.tile([128, 512], F32, tag="py", bufs=2)
                              nc.tensor.matmul(y_ps[:], wo_t[0][:, dm * 128:(dm + 1) * 128],
                                               out_q[0][:], start=True, stop=False)
                              nc.tensor.matmul(y_ps[:], wo_t[1][:, dm * 128:(dm + 1) * 128],
                                               out_q[1][:], start=False, stop=True)
                              y_sb = ap.tile([128, 512], BF16 if V4_YBF16 else F32, tag="y_sb", bufs=3)
                              if dm % 4 == 0:
                                  nc.scalar.activation(y_sb[:], y_ps[:], ACTF.Identity)
                              else:
                                  nc.vector.tensor_copy(y_sb[:], y_ps[:])
                              nc.sync.dma_start(
                                  yT_d.ap()[dm * 128:(dm + 1) * 128, b * S + q0: b * S + q0 + 512],
                                  y_sb[:])

            secs = {"pass1": sec_pass1, "main": sec_main, "pass3": sec_pass3, "attn": sec_attn}
            if phase == "full":
                for rep in range(reps):
                    for s in ("pass1", "main", "pass3", "attn"):
                        secs[s]()
            else:
                for s in ("pass1", "main", "pass3", "attn"):
                    for rep in range(reps if s == phase else 1):
                        secs[s]()

    nc.finalize()
    _BUILD_CACHE[("nc", reps, phase)] = nc
    return nc


def _host_inputs(x, wq_down, q_norm_w, wq_up, wq_rope, wkv_down, kv_norm_w, wkv_up, wk_rope, wo):
    """Build the 8 per-core input maps."""
    f32 = np.float32
    x = np.asarray(x, f32)
    xT = np.ascontiguousarray(np.transpose(x, (0, 2, 1))).astype(NPBF16)   # [B, DIM, S]

    p64 = np.concatenate([np.arange(0, DR, 2), np.arange(1, DR, 2)])  # deinterleave

    wq_down_n = (np.asarray(q_norm_w, f32)[:, None] * np.asarray(wq_down, f32))  # [QR, DIM]
    wkv_up_eff = np.asarray(wkv_up, f32) * np.asarray(kv_norm_w, f32)[None, :]   # [NH*(DN+DV), KVR]

    # rope tables (deinterleaved convention), stacked x2 for the two heads
    inv_freq = (1.0 / (ROPE_THETA ** (np.arange(0, DR, 2, dtype=np.float64) / DR)))  # [32]
    ang = np.arange(S, dtype=np.float64)[:, None] * inv_freq[None, :]                # [S, 32]
    cos_t, sin_t = np.cos(ang), np.sin(ang)
    C64 = np.concatenate([cos_t.T, cos_t.T], axis=0).astype(f32)                     # [64, S]
    S64 = np.concatenate([-sin_t.T, sin_t.T], axis=0).astype(f32)                    # [64, S]
    ctab = np.concatenate([C64, C64], axis=0)                                        # [128, S]
    stab = np.concatenate([S64, S64], axis=0)

    # within-128-block causal mask (k partition p valid for local q col >= p)
    kr_i = np.arange(128)[:, None]
    qr_i = np.arange(128)[None, :]
    masks = (kr_i <= qr_i).astype(NPBF16)

    ident = np.eye(128, dtype=f32).astype(NPBF16)
    ones = np.ones((128, 1), dtype=NPBF16)

    in_maps = []
    for c in range(N_CORES):
        h0, h1 = HPC * c, HPC * c + 1
        bo, so = c // 4, (c % 4) * 512
        wq_blocks, wqp_blocks, wkvu_cols, wo_list = [], [], [], []
        for h in (h0, h1):
            wq_blocks.append(np.asarray(wq_up, f32)[h * DN:(h + 1) * DN, :] @ wq_down_n)
            wr = np.asarray(wq_rope, f32)[h * DR:(h + 1) * DR, :][p64, :]
            wqp_blocks.append(wr @ wq_down_n)
            wkvu_cols.append(wkv_up_eff[h * (DN + DV): h * (DN + DV) + DN, :].T)      # K_h  [KVR, DN]
            wkvu_cols.append(wkv_up_eff[h * (DN + DV) + DN: (h + 1) * (DN + DV), :].T)  # V_h
            wo_list.append(np.ascontiguousarray(np.asarray(wo, f32)[:, h * DV:(h + 1) * DV].T))
        in_maps.append({
            "xT": xT,
            "xo": np.ascontiguousarray(xT[bo, :, so:so + OWN]),
            "wqss": np.ascontiguousarray(np.asarray(wq_down, f32)[c * RSH:(c + 1) * RSH, :].T).astype(NPBF16),
            "wq": np.ascontiguousarray(np.concatenate(wq_blocks, axis=0).T).astype(NPBF16),
            "wqp": np.ascontiguousarray(np.concatenate(wqp_blocks, axis=0).T).astype(NPBF16),
            "wkvd": np.ascontiguousarray(np.asarray(wkv_down, f32).T).astype(NPBF16),
            "wkvu": np.ascontiguousarray(np.concatenate(wkvu_cols, axis=1)).astype(NPBF16),
            "wkr": np.ascontiguousarray(np.asarray(wk_rope, f32)[p64, :].T).astype(NPBF16),
            "wo0": wo_list[0].astype(NPBF16),
            "wo1": wo_list[1].astype(NPBF16),
            "ctab": ctab,
            "stab": stab,
            "ctabo": np.ascontiguousarray(ctab[0:64, so:so + OWN]),
            "stabo": np.ascontiguousarray(stab[0:64, so:so + OWN]),
            "masks": masks,
            "ident": ident,
            "ones": ones,
        })
    return in_maps


def kernel(**inputs) -> np.ndarray:
    nc = _build_program(1)
    in_maps = _host_inputs(**inputs)
    res = run_bass_kernel_spmd(nc, in_maps, core_ids=list(range(N_CORES)))
    yT = np.zeros((DIM, T), np.float32)
    for c in range(N_CORES):
        yT += res.results[c]["yT"].astype(np.float32)
    return np.ascontiguousarray(yT.T.reshape(B, S, DIM))
